# revision 4
# baseline (speedup 1.0000x reference)
"""TRN2 Bass kernel for nn_Decoder_25615184954195 (2-layer LSTM decoder, 32 steps).

Strategy: data-parallel over batch (8 cores x 32 rows), full sequential loop
local per core.  bf16 matmuls with fp32 PSUM accumulation; weights bf16 with
21 of 32 k-tiles SBUF-resident and 11 streamed from HBM each step through a
3-slot rotating pool.  The column-tiled PE runs at ~51 ns/MM (4x concurrency),
so the kernel is bound by the per-step weight-streaming HBM traffic.

Per-core layouts:
  - Contraction K = 2048 = [x(1024) | h(1024)], 16 k-tiles of 128.
  - Stationary operand: xhT k-tile [128, 32] bf16 (batch in the free dim).
  - Moving operand: column-permuted weight k-tile [128, 4096] bf16; the 4-way
    column-tiled matmul's col-group j computes H-quarter j for all 4 gates:
    psum[32*j + b, 256*G + h2] = gates[b, 1024*G + 256*j + h2].
  - Folded elementwise layout: partitions = (H-quarter, batch) so gate math
    runs on all 128 lanes.  c stays fp32.  h is rebuilt transposed via two
    PE transposes, deferred into the next layer-step between its h-rounds and
    x-rounds so the PE fills the gate-chain latency with matmuls.
"""
import sys

for _p in ("/opt/trn_rl_repo", "/root/.axon_site/_ro/trn_rl_repo"):
    if _p not in sys.path:
        sys.path.insert(0, _p)

import numpy as np
import ml_dtypes

import concourse.bacc as bacc
import concourse.mybir as mybir
import concourse.tile as tile
from concourse import masks
from concourse import bass_utils

BF16 = mybir.dt.bfloat16
F32 = mybir.dt.float32
AF = mybir.ActivationFunctionType

N_CORES = 8
B = 32            # batch rows per core (256 total)
KT = 16           # k-tiles per layer
L = 2
GCOLS = 4096
NUM_STEPS = 32
N_STREAM = 11     # weight k-tiles streamed per step (rest SBUF-resident)
N_SLOTS = 3


def _stream_tiles(n_stream):
    order = []
    for k in range(KT // 2 - 1, -1, -1):
        for l in (1, 0):
            order.append((l, k))
    return set(order[:n_stream])


def _build(num_steps=NUM_STEPS, n_stream=N_STREAM, n_slots=N_SLOTS, reps=1):
    stream_set = _stream_tiles(n_stream)
    def round_pos(lk):
        l, k = lk
        r = k - 8 if k >= 8 else 8 + k
        return l * KT + r

    # resident tiles load at startup in first-use order so step 0 starts ASAP
    res_list = sorted(((l, k) for l in range(L) for k in range(KT)
                       if (l, k) not in stream_set), key=round_pos)
    str_list = sorted(stream_set, key=round_pos)

    nc = bacc.Bacc("TRN2", target_bir_lowering=False, debug=False,
                   num_devices=N_CORES)

    d_wres = nc.dram_tensor("wres", [max(len(res_list), 1), 128, GCOLS], BF16,
                            kind="ExternalInput")
    d_wstr = nc.dram_tensor("wstr", [max(len(str_list), 1), 128, GCOLS], BF16,
                            kind="ExternalInput")
    d_bias = nc.dram_tensor("biasf", [L, 128, 1024], BF16, kind="ExternalInput")
    d_x0t = nc.dram_tensor("x0t", [128, 8 * B], BF16, kind="ExternalInput")
    d_h0t = nc.dram_tensor("h0t", [L, 2, 128, 128], BF16, kind="ExternalInput")
    d_c0 = nc.dram_tensor("c0f", [L, 128, 256], F32, kind="ExternalInput")
    d_wlin = nc.dram_tensor("wlint", [128, 8 * 512], BF16, kind="ExternalInput")
    d_blin = nc.dram_tensor("blinr", [B, 512], F32, kind="ExternalInput")
    d_y = nc.dram_tensor("y", [B, 512], F32, kind="ExternalOutput")

    with tile.TileContext(nc) as tc:
        import contextlib
        with contextlib.ExitStack() as ctx:
            sb = ctx.enter_context(tc.tile_pool(name="sb", bufs=1))
            sb2 = ctx.enter_context(tc.tile_pool(name="sb2", bufs=2))
            psum = ctx.enter_context(tc.tile_pool(name="psum", bufs=2, space="PSUM"))
            psum1 = ctx.enter_context(tc.tile_pool(name="psum1", bufs=2, space="PSUM"))
            psumgb = ctx.enter_context(tc.tile_pool(name="psumgb", bufs=1, space="PSUM"))

            w_sb = {}
            for i, (l, k) in enumerate(res_list):
                t = sb.tile([128, GCOLS], BF16, name=f"w_{l}_{k}")
                nc.sync.dma_start(t[:], d_wres[i])
                w_sb[(l, k)] = t
            slots = [sb.tile([128, GCOLS], BF16, name=f"slot{i}")
                     for i in range(n_slots)]
            bias_sb = sb.tile([128, L * 1024], BF16, name="bias_sb")
            for l in range(L):
                nc.sync.dma_start(bias_sb[:, 1024 * l:1024 * (l + 1)], d_bias[l])
            x0t_sb = sb.tile([128, 8 * B], BF16, name="x0t_sb")
            nc.sync.dma_start(x0t_sb[:], d_x0t[:])
            hT = [[sb.tile([128, 128], BF16, name=f"hT_{l}_{hf}") for hf in range(2)]
                  for l in range(L)]
            for l in range(L):
                for hf in range(2):
                    nc.sync.dma_start(hT[l][hf][:], d_h0t[l, hf])
            c_sb = [sb.tile([128, 256], F32, name=f"c_{l}") for l in range(L)]
            for l in range(L):
                nc.sync.dma_start(c_sb[l][:], d_c0[l])
            ident = sb.tile([128, 128], BF16, name="ident")
            masks.make_identity(nc, ident[:])

            # Streamed-tile software pipeline: use #u lives in slots[u % n_slots];
            # the DMA for use u+n_slots is emitted right after use u's matmuls so
            # Tile's emission-order dependency tracking yields DMA->MMs->DMA->MMs
            # per slot with n_slots of prefetch depth.
            stream_uses = []
            for _t in range(num_steps):
                for _l in range(L):
                    for _k in (list(range(8, 16)) + list(range(0, 8))):
                        if (_l, _k) in stream_set:
                            stream_uses.append((_t, _l, _k))
            use_idx = {u_lk: u for u, u_lk in enumerate(stream_uses)}

            def emit_stream_dma(u):
                if u >= len(stream_uses):
                    if reps == 1:
                        return
                    u = u % len(stream_uses)  # wrap prefetch for For_i timing reps
                _t, _l, _k = stream_uses[u]
                nc.sync.dma_start(slots[u % n_slots][:],
                                  d_wstr[str_list.index((_l, _k))])

            for u in range(min(n_slots, len(stream_uses))):
                emit_stream_dma(u)

            def stat_ap(l, k, t):
                if k >= 8:
                    kh = k - 8
                    return hT[l][kh % 2][:, 32 * (kh // 2):32 * (kh // 2) + 32]
                if l == 0:
                    if t == 0:
                        return x0t_sb[:, 32 * k:32 * (k + 1)]
                    src = hT[1]
                else:
                    src = hT[0]
                return src[k % 2][:, 32 * (k // 2):32 * (k // 2) + 32]

            ROUNDS = list(range(8, 16)) + list(range(0, 8))

            pending_tr = []   # deferred transposes: (hbf tile, target layer)

            def flush_transposes():
                while pending_tr:
                    hbf_p, l_p = pending_tr.pop(0)
                    for hf in range(2):
                        pt = psum1.tile([128, 128], BF16, name="pt", tag="pt")
                        nc.tensor.transpose(pt[:], hbf_p[:, 128 * hf:128 * (hf + 1)],
                                            ident[:])
                        nc.vector.tensor_copy(hT[l_p][hf][:], pt[:])

            rep_cm = tc.For_i(0, reps) if reps > 1 else contextlib.nullcontext()
            with rep_cm:
              for t in range(num_steps):
                for l in range(L):
                    ps = psum.tile([128, 1024], F32, name="gps", tag="gps")
                    for ri, k in enumerate(ROUNDS):
                        if ri == 8:
                            flush_transposes()   # before x-rounds need hT
                        if (l, k) in w_sb:
                            w = w_sb[(l, k)]
                        else:
                            w = slots[use_idx[(t, l, k)] % n_slots]
                        lhsT = stat_ap(l, k, t)
                        for hv in range(2):      # PSUM bank limit: N<=512 fp32
                            for j in range(4):   # col-groups run concurrently
                                nc.tensor.matmul(
                                    ps[32 * j:32 * (j + 1), 512 * hv:512 * (hv + 1)],
                                    lhsT,
                                    w[:, 1024 * j + 512 * hv:1024 * j + 512 * (hv + 1)],
                                    start=(ri == 0), stop=(ri == len(ROUNDS) - 1),
                                    tile_position=(0, 32 * j),
                                    skip_group_check=True,
                                )
                        if (l, k) not in w_sb:
                            emit_stream_dma(use_idx[(t, l, k)] + n_slots)

                    gb = psumgb.tile([128, 1024], F32, name="gb", tag="gb")
                    nc.vector.tensor_add(gb[:], ps[:],
                                         bias_sb[:, 1024 * l:1024 * (l + 1)])
                    s_if = sb.tile([128, 512], F32, name="s_if", tag="s_if")
                    t_g = sb.tile([128, 256], F32, name="t_g", tag="t_g")
                    s_o = sb.tile([128, 256], F32, name="s_o", tag="s_o")
                    nc.scalar.activation(s_if[:], gb[:, 0:512], AF.Sigmoid)
                    nc.scalar.activation(t_g[:], gb[:, 512:768], AF.Tanh)
                    nc.scalar.activation(s_o[:], gb[:, 768:1024], AF.Sigmoid)
                    t1 = sb.tile([128, 256], F32, name="t1", tag="t1")
                    t2 = sb.tile([128, 256], F32, name="t2", tag="t2")
                    nc.vector.tensor_mul(t1[:], s_if[:, 256:512], c_sb[l][:])
                    nc.vector.tensor_mul(t2[:], s_if[:, 0:256], t_g[:])
                    nc.vector.tensor_add(c_sb[l][:], t1[:], t2[:])
                    th = sb.tile([128, 256], F32, name="th", tag="t1")
                    nc.scalar.activation(th[:], c_sb[l][:], AF.Tanh)
                    hbf = sb2.tile([128, 256], BF16, name="hbf", tag="hbf")
                    nc.vector.tensor_mul(hbf[:], s_o[:], th[:])
                    pending_tr.append((hbf, l))

            flush_transposes()

            # ---- linear head: pred = h1 @ W_lin.T + b_lin ----
            wlin_slot = slots[len(stream_uses) % n_slots]
            nc.sync.dma_start(wlin_slot[:], d_wlin[:])
            blin_sb = sb.tile([B, 512], F32, name="blin_sb", tag="t1")
            nc.sync.dma_start(blin_sb[:], d_blin[:])
            ph = psum.tile([B, 512], F32, name="ph", tag="gps")
            for k in range(8):
                lhsT = hT[1][k % 2][:, 32 * (k // 2):32 * (k // 2) + 32]
                nc.tensor.matmul(ph[:], lhsT, wlin_slot[:, 512 * k:512 * (k + 1)],
                                 start=(k == 0), stop=(k == 7))
            out_sb = sb.tile([B, 512], F32, name="out_sb", tag="s_if")
            nc.vector.tensor_add(out_sb[:], ph[:], blin_sb[:])
            nc.sync.dma_start(d_y[:], out_sb[:])

    nc.compile()
    return nc, dict(res_list=res_list, str_list=str_list)


def _prep_common(inputs, meta):
    W_ih, W_hh = np.asarray(inputs["W_ih"]), np.asarray(inputs["W_hh"])
    b_sum = np.asarray(inputs["b_ih"]) + np.asarray(inputs["b_hh"])
    Wcat = np.concatenate([W_ih, W_hh], axis=2)               # [L, 4096, 2048]
    A = Wcat.reshape(L, 4, 4, 256, KT, 128)                   # [l, G, j, h2, k, p]
    Wt = np.ascontiguousarray(A.transpose(0, 4, 5, 2, 1, 3)   # [l, k, p, j, G, h2]
                              ).reshape(L, KT, 128, GCOLS).astype(ml_dtypes.bfloat16)
    Bf = b_sum.reshape(L, 4, 4, 256).transpose(0, 2, 1, 3)    # [l, j, G, h2]
    bias_fold = np.broadcast_to(Bf.reshape(L, 4, 1, 1024), (L, 4, 32, 1024))
    bias_fold = np.ascontiguousarray(bias_fold).reshape(L, 128, 1024)
    bias_fold = bias_fold.astype(ml_dtypes.bfloat16)
    wres = (np.stack([Wt[l, k] for (l, k) in meta["res_list"]])
            if meta["res_list"] else np.zeros((1, 128, GCOLS), ml_dtypes.bfloat16))
    wstr = (np.stack([Wt[l, k] for (l, k) in meta["str_list"]])
            if meta["str_list"] else np.zeros((1, 128, GCOLS), ml_dtypes.bfloat16))
    W_lin = np.asarray(inputs["W_lin"])
    wlint = np.ascontiguousarray(
        W_lin.T.reshape(8, 128, 512).transpose(1, 0, 2).reshape(128, 8 * 512)
    ).astype(ml_dtypes.bfloat16)
    blin = np.ascontiguousarray(np.broadcast_to(
        np.asarray(inputs["b_lin"]).astype(np.float32)[None, :], (B, 512)))
    return dict(wres=wres, wstr=wstr, biasf=bias_fold, wlint=wlint, blinr=blin)


def _prep_core(inputs, ci):
    s = slice(ci * B, (ci + 1) * B)
    x = np.asarray(inputs["input_seq"])[s]
    h = np.asarray(inputs["h"])[:, s]
    c = np.asarray(inputs["c"])[:, s]
    x0t = np.ascontiguousarray(
        x.reshape(B, 8, 128).transpose(2, 1, 0).reshape(128, 8 * B)
    ).astype(ml_dtypes.bfloat16)
    hr = h.reshape(L, B, 4, 2, 128)
    h0t = np.ascontiguousarray(hr.transpose(0, 3, 4, 2, 1)
                               ).reshape(L, 2, 128, 128).astype(ml_dtypes.bfloat16)
    cr = c.reshape(L, B, 4, 256).transpose(0, 2, 1, 3)
    c0f = np.ascontiguousarray(cr).reshape(L, 128, 256).astype(np.float32)
    return dict(x0t=x0t, h0t=h0t, c0f=c0f)


_CACHE = {}


def _get_built():
    if "nc" not in _CACHE:
        _CACHE["nc"], _CACHE["meta"] = _build()
    return _CACHE["nc"], _CACHE["meta"]


def kernel(**inputs) -> np.ndarray:
    nc, meta = _get_built()
    common = _prep_common(inputs, meta)
    in_maps = [dict(common, **_prep_core(inputs, ci)) for ci in range(N_CORES)]
    r = bass_utils.run_bass_kernel_spmd(nc, in_maps, core_ids=list(range(N_CORES)))
    y = np.concatenate([r.results[ci]["y"] for ci in range(N_CORES)], axis=0)
    return y.astype(np.float32)



# revision 11
# speedup vs baseline: 1.1945x; 1.1945x over previous
"""TRN2 Bass kernel for nn_Decoder_25615184954195 (2-layer LSTM decoder, 32 steps).

Strategy: data-parallel over batch (8 cores x 32 rows), full sequential loop
local per core.  bf16 matmuls with fp32 PSUM accumulation; weights bf16 with
21 of 32 k-tiles SBUF-resident and 11 streamed from HBM each step through a
3-slot rotating pool.  The column-tiled PE runs at ~51 ns/MM (4x concurrency),
so the kernel is bound by the per-step weight-streaming HBM traffic.

Per-core layouts:
  - Contraction K = 2048 = [x(1024) | h(1024)], 16 k-tiles of 128.
  - Stationary operand: xhT k-tile [128, 32] bf16 (batch in the free dim).
  - Moving operand: column-permuted weight k-tile [128, 4096] bf16; the 4-way
    column-tiled matmul's col-group j computes H-quarter j for all 4 gates:
    psum[32*j + b, 256*G + h2] = gates[b, 1024*G + 256*j + h2].
  - Folded elementwise layout: partitions = (H-quarter, batch) so gate math
    runs on all 128 lanes.  c stays fp32.  h is rebuilt transposed via two
    PE transposes, deferred into the next layer-step between its h-rounds and
    x-rounds so the PE fills the gate-chain latency with matmuls.
"""
import sys

for _p in ("/opt/trn_rl_repo", "/root/.axon_site/_ro/trn_rl_repo"):
    if _p not in sys.path:
        sys.path.insert(0, _p)

import numpy as np
import ml_dtypes

import concourse.bacc as bacc
import concourse.mybir as mybir
import concourse.tile as tile
from concourse import masks
from concourse import bass_utils

BF16 = mybir.dt.bfloat16
F32 = mybir.dt.float32
AF = mybir.ActivationFunctionType

N_CORES = 8
B = 32            # batch rows per core (256 total)
KT = 16           # k-tiles per layer
L = 2
GCOLS = 4096
NUM_STEPS = 32
N_STREAM = 0      # weight k-tiles streamed per step (rest SBUF-resident)
N_SLOTS = 1
WDT = mybir.dt.float8e3          # weight storage: fp8 e3m4, scaled
WDT_NP = ml_dtypes.float8_e3m4
WSCALE = 64.0                    # weights stored as w*WSCALE in fp8


def _stream_tiles(n_stream):
    order = []
    for k in range(KT // 2 - 1, -1, -1):
        for l in (1, 0):
            order.append((l, k))
    return set(order[:n_stream])


def _build(num_steps=NUM_STEPS, n_stream=N_STREAM, n_slots=N_SLOTS, reps=1):
    stream_set = _stream_tiles(n_stream)
    def round_pos(lk):
        l, k = lk
        r = k - 8 if k >= 8 else 8 + k
        return l * KT + r

    # resident tiles load at startup in first-use order so step 0 starts ASAP
    res_list = sorted(((l, k) for l in range(L) for k in range(KT)
                       if (l, k) not in stream_set), key=round_pos)
    str_list = sorted(stream_set, key=round_pos)

    nc = bacc.Bacc("TRN2", target_bir_lowering=False, debug=False,
                   num_devices=N_CORES)

    d_wres = nc.dram_tensor("wres", [max(len(res_list), 1), 128, GCOLS], WDT,
                            kind="ExternalInput")
    d_wstr = nc.dram_tensor("wstr", [max(len(str_list), 1), 128, GCOLS], WDT,
                            kind="ExternalInput")
    d_bias = nc.dram_tensor("biasf", [L, 128, 1024], BF16, kind="ExternalInput")
    d_x0t = nc.dram_tensor("x0t", [128, 8 * B], BF16, kind="ExternalInput")
    d_h0t = nc.dram_tensor("h0t", [L, 2, 128, 128], BF16, kind="ExternalInput")
    d_c0 = nc.dram_tensor("c0f", [L, 128, 256], F32, kind="ExternalInput")
    d_wlin = nc.dram_tensor("wlint", [128, 8 * 512], BF16, kind="ExternalInput")
    d_blin = nc.dram_tensor("blinr", [B, 512], F32, kind="ExternalInput")
    d_y = nc.dram_tensor("y", [B, 512], F32, kind="ExternalOutput")

    with tile.TileContext(nc) as tc:
        import contextlib
        with contextlib.ExitStack() as ctx:
            sb = ctx.enter_context(tc.tile_pool(name="sb", bufs=1))
            sb2 = ctx.enter_context(tc.tile_pool(name="sb2", bufs=2))
            psum = ctx.enter_context(tc.tile_pool(name="psum", bufs=2, space="PSUM"))
            psum1 = ctx.enter_context(tc.tile_pool(name="psum1", bufs=2, space="PSUM"))
            psumgb = ctx.enter_context(tc.tile_pool(name="psumgb", bufs=1, space="PSUM"))

            w_sb = {}
            for i, (l, k) in enumerate(res_list):
                t = sb.tile([128, GCOLS], WDT, name=f"w_{l}_{k}")
                nc.sync.dma_start(t[:], d_wres[i])
                w_sb[(l, k)] = t
            slots = [sb.tile([128, GCOLS], WDT, name=f"slot{i}")
                     for i in range(n_slots)]
            wlin_sb = sb.tile([128, 8 * 512], BF16, name="wlin_sb")
            nc.sync.dma_start(wlin_sb[:], d_wlin[:])
            bias_sb = sb.tile([128, L * 1024], BF16, name="bias_sb")
            for l in range(L):
                nc.sync.dma_start(bias_sb[:, 1024 * l:1024 * (l + 1)], d_bias[l])
            x0t_sb = sb.tile([128, 8 * B], BF16, name="x0t_sb")
            nc.sync.dma_start(x0t_sb[:], d_x0t[:])
            hT = [[sb.tile([128, 128], BF16, name=f"hT_{l}_{hf}") for hf in range(2)]
                  for l in range(L)]
            for l in range(L):
                for hf in range(2):
                    nc.sync.dma_start(hT[l][hf][:], d_h0t[l, hf])
            c_sb = [sb.tile([128, 256], F32, name=f"c_{l}") for l in range(L)]
            for l in range(L):
                nc.sync.dma_start(c_sb[l][:], d_c0[l])
            ident = sb.tile([128, 128], BF16, name="ident")
            masks.make_identity(nc, ident[:])

            # Streamed-tile software pipeline: use #u lives in slots[u % n_slots];
            # the DMA for use u+n_slots is emitted right after use u's matmuls so
            # Tile's emission-order dependency tracking yields DMA->MMs->DMA->MMs
            # per slot with n_slots of prefetch depth.
            stream_uses = []
            for _t in range(num_steps):
                for _l in range(L):
                    for _k in (list(range(8, 16)) + list(range(0, 8))):
                        if (_l, _k) in stream_set:
                            stream_uses.append((_t, _l, _k))
            use_idx = {u_lk: u for u, u_lk in enumerate(stream_uses)}

            def emit_stream_dma(u):
                if u >= len(stream_uses):
                    if reps == 1:
                        return
                    u = u % len(stream_uses)  # wrap prefetch for For_i timing reps
                _t, _l, _k = stream_uses[u]
                nc.sync.dma_start(slots[u % n_slots][:],
                                  d_wstr[str_list.index((_l, _k))])

            for u in range(min(n_slots, len(stream_uses))):
                emit_stream_dma(u)

            def stat_ap(l, k, t):
                if k >= 8:
                    kh = k - 8
                    return hT[l][kh % 2][:, 32 * (kh // 2):32 * (kh // 2) + 32]
                if l == 0:
                    if t == 0:
                        return x0t_sb[:, 32 * k:32 * (k + 1)]
                    src = hT[1]
                else:
                    src = hT[0]
                return src[k % 2][:, 32 * (k // 2):32 * (k // 2) + 32]

            ROUNDS = list(range(8, 16)) + list(range(0, 8))

            pending_tr = []   # deferred transposes: (hbf tile, target layer)

            def flush_transposes():
                while pending_tr:
                    hbf_p, l_p = pending_tr.pop(0)
                    for hf in range(2):
                        pt = psum1.tile([128, 128], BF16, name="pt", tag="pt")
                        nc.tensor.transpose(pt[:], hbf_p[:, 128 * hf:128 * (hf + 1)],
                                            ident[:])
                        nc.vector.tensor_copy(hT[l_p][hf][:], pt[:])

            rep_cm = tc.For_i(0, reps) if reps > 1 else contextlib.nullcontext()
            with rep_cm:
              for t in range(num_steps):
                for l in range(L):
                    ps = psum.tile([128, 1024], F32, name="gps", tag="gps")
                    for ri, k in enumerate(ROUNDS):
                        if ri == 8:
                            flush_transposes()   # before x-rounds need hT
                        if (l, k) in w_sb:
                            w = w_sb[(l, k)]
                        else:
                            w = slots[use_idx[(t, l, k)] % n_slots]
                        lhsT = stat_ap(l, k, t)
                        for hv in range(2):      # PSUM bank limit: N<=512 fp32
                            for j in range(4):   # col-groups run concurrently
                                nc.tensor.matmul(
                                    ps[32 * j:32 * (j + 1), 512 * hv:512 * (hv + 1)],
                                    lhsT,
                                    w[:, 1024 * j + 512 * hv:1024 * j + 512 * (hv + 1)],
                                    start=(ri == 0), stop=(ri == len(ROUNDS) - 1),
                                    tile_position=(0, 32 * j),
                                    skip_group_check=True,
                                )
                        if (l, k) not in w_sb:
                            emit_stream_dma(use_idx[(t, l, k)] + n_slots)

                    gb = psumgb.tile([128, 1024], F32, name="gb", tag="gb")
                    nc.vector.tensor_add(gb[:], ps[:],
                                         bias_sb[:, 1024 * l:1024 * (l + 1)])
                    s_if = sb.tile([128, 512], F32, name="s_if", tag="s_if")
                    t_g = sb.tile([128, 256], F32, name="t_g", tag="t_g")
                    s_o = sb.tile([128, 256], F32, name="s_o", tag="s_o")
                    nc.scalar.activation(s_if[:], gb[:, 0:512], AF.Sigmoid,
                                         scale=1.0 / WSCALE)
                    nc.scalar.activation(t_g[:], gb[:, 512:768], AF.Tanh,
                                         scale=1.0 / WSCALE)
                    nc.scalar.activation(s_o[:], gb[:, 768:1024], AF.Sigmoid,
                                         scale=1.0 / WSCALE)
                    t1 = sb.tile([128, 256], F32, name="t1", tag="t1")
                    t2 = sb.tile([128, 256], F32, name="t2", tag="t2")
                    nc.vector.tensor_mul(t1[:], s_if[:, 256:512], c_sb[l][:])
                    nc.vector.tensor_mul(t2[:], s_if[:, 0:256], t_g[:])
                    nc.vector.tensor_add(c_sb[l][:], t1[:], t2[:])
                    th = sb.tile([128, 256], F32, name="th", tag="t1")
                    nc.scalar.activation(th[:], c_sb[l][:], AF.Tanh)
                    hbf = sb2.tile([128, 256], BF16, name="hbf", tag="hbf")
                    nc.vector.tensor_mul(hbf[:], s_o[:], th[:])
                    pending_tr.append((hbf, l))

            flush_transposes()

            # ---- linear head: pred = h1 @ W_lin.T + b_lin ----
            blin_sb = sb.tile([B, 512], F32, name="blin_sb", tag="t1")
            nc.sync.dma_start(blin_sb[:], d_blin[:])
            ph = psum.tile([B, 512], F32, name="ph", tag="gps")
            for k in range(8):
                lhsT = hT[1][k % 2][:, 32 * (k // 2):32 * (k // 2) + 32]
                nc.tensor.matmul(ph[:], lhsT, wlin_sb[:, 512 * k:512 * (k + 1)],
                                 start=(k == 0), stop=(k == 7))
            out_sb = sb.tile([B, 512], F32, name="out_sb", tag="s_if")
            nc.vector.tensor_add(out_sb[:], ph[:], blin_sb[:])
            nc.sync.dma_start(d_y[:], out_sb[:])

    nc.compile()
    return nc, dict(res_list=res_list, str_list=str_list)


def _prep_common(inputs, meta):
    W_ih, W_hh = np.asarray(inputs["W_ih"]), np.asarray(inputs["W_hh"])
    b_sum = np.asarray(inputs["b_ih"]) + np.asarray(inputs["b_hh"])
    Wcat = np.concatenate([W_ih, W_hh], axis=2)               # [L, 4096, 2048]
    A = Wcat.reshape(L, 4, 4, 256, KT, 128)                   # [l, G, j, h2, k, p]
    Wt = np.ascontiguousarray(A.transpose(0, 4, 5, 2, 1, 3)   # [l, k, p, j, G, h2]
                              ).reshape(L, KT, 128, GCOLS)
    Wt = (Wt * WSCALE).astype(WDT_NP)
    Bf = b_sum.reshape(L, 4, 4, 256).transpose(0, 2, 1, 3)    # [l, j, G, h2]
    bias_fold = np.broadcast_to(Bf.reshape(L, 4, 1, 1024), (L, 4, 32, 1024))
    bias_fold = np.ascontiguousarray(bias_fold).reshape(L, 128, 1024)
    bias_fold = (bias_fold * WSCALE).astype(ml_dtypes.bfloat16)
    wres = (np.stack([Wt[l, k] for (l, k) in meta["res_list"]])
            if meta["res_list"] else np.zeros((1, 128, GCOLS), WDT_NP))
    wstr = (np.stack([Wt[l, k] for (l, k) in meta["str_list"]])
            if meta["str_list"] else np.zeros((1, 128, GCOLS), WDT_NP))
    W_lin = np.asarray(inputs["W_lin"])
    wlint = np.ascontiguousarray(
        W_lin.T.reshape(8, 128, 512).transpose(1, 0, 2).reshape(128, 8 * 512)
    ).astype(ml_dtypes.bfloat16)
    blin = np.ascontiguousarray(np.broadcast_to(
        np.asarray(inputs["b_lin"]).astype(np.float32)[None, :], (B, 512)))
    return dict(wres=wres, wstr=wstr, biasf=bias_fold, wlint=wlint, blinr=blin)


def _prep_core(inputs, ci):
    s = slice(ci * B, (ci + 1) * B)
    x = np.asarray(inputs["input_seq"])[s]
    h = np.asarray(inputs["h"])[:, s]
    c = np.asarray(inputs["c"])[:, s]
    x0t = np.ascontiguousarray(
        x.reshape(B, 8, 128).transpose(2, 1, 0).reshape(128, 8 * B)
    ).astype(ml_dtypes.bfloat16)
    hr = h.reshape(L, B, 4, 2, 128)
    h0t = np.ascontiguousarray(hr.transpose(0, 3, 4, 2, 1)
                               ).reshape(L, 2, 128, 128).astype(ml_dtypes.bfloat16)
    cr = c.reshape(L, B, 4, 256).transpose(0, 2, 1, 3)
    c0f = np.ascontiguousarray(cr).reshape(L, 128, 256).astype(np.float32)
    return dict(x0t=x0t, h0t=h0t, c0f=c0f)


_CACHE = {}


def _get_built():
    if "nc" not in _CACHE:
        _CACHE["nc"], _CACHE["meta"] = _build()
    return _CACHE["nc"], _CACHE["meta"]


def kernel(**inputs) -> np.ndarray:
    nc, meta = _get_built()
    common = _prep_common(inputs, meta)
    in_maps = [dict(common, **_prep_core(inputs, ci)) for ci in range(N_CORES)]
    r = bass_utils.run_bass_kernel_spmd(nc, in_maps, core_ids=list(range(N_CORES)))
    y = np.concatenate([r.results[ci]["y"] for ci in range(N_CORES)], axis=0)
    return y.astype(np.float32)



# revision 29
# speedup vs baseline: 2.4186x; 2.0247x over previous
"""TRN2 Bass kernel for nn_Decoder_25615184954195 (2-layer LSTM decoder, 32 steps).

Strategy: data-parallel over batch (8 cores x 32 rows), full sequential loop
local per core.  All weights SBUF-resident in fp8 e4m3 (x128 scale), paired
into DoubleRow matmul tiles (K=256 per instruction) so the PE streams two
fp8 weight elements per lane-cycle: the kernel is bound by PE weight-column
throughput (batch 32 fills only a quarter of the 128-wide array), so halving
the streamed column count halves the dominant cost.  Activations (x, h
feedback) are quantized to fp8 e4m3 (x32 scale) each step; the scale product
4096 folds into the activation scale, and the linear head's weights are
pre-divided by 32 on the host so no extra compensation op is needed.

DoubleRow forbids PE column-tiling (tile_position col offset must be 0), so
gates live in PSUM as [32, 4096] split into two [32, 2048] halves A=(i,f),
B=(g,o) (4 banks each, double-buffered across layer-steps = all 8 banks).
Gate math runs on 32 partitions.  h is transposed for the next contraction
by the DMA XBAR (dma_start transpose=True, bf16) instead of PE transposes,
then quantized to fp8 by one Act-engine Copy(scale=32); MM rounds are
ordered h-pairs first / x-pairs last so the transpose+quant latency hides
under the next layer-step's h-pair matmuls.
"""
import sys

for _p in ("/opt/trn_rl_repo", "/root/.axon_site/_ro/trn_rl_repo"):
    if _p not in sys.path:
        sys.path.insert(0, _p)

import numpy as np
import ml_dtypes

import concourse.bacc as bacc
import concourse.mybir as mybir
import concourse.tile as tile
from concourse import bass_utils

BF16 = mybir.dt.bfloat16
F32 = mybir.dt.float32
F8 = mybir.dt.float8e4
F8_NP = ml_dtypes.float8_e4m3
AF = mybir.ActivationFunctionType
DR = mybir.MatmulPerfMode.DoubleRow

N_CORES = 8
B = 32            # batch rows per core (256 total)
KP = 8            # DoubleRow k-pair tiles per layer (K = 2048 = 8 * 256)
L = 2
GCOLS = 4096
NUM_STEPS = 32
WSCALE = 128.0    # weights stored as w*WSCALE in fp8 e4m3
XSCALE = 32.0     # x/h stationary stored as v*XSCALE in fp8 e4m3
ASCALE = WSCALE * XSCALE
# kept for test.py compat (unused)
N_STREAM = 0
N_SLOTS = 1


def _build(num_steps=NUM_STEPS, n_stream=N_STREAM, n_slots=N_SLOTS, reps=1):
    # kp 4..7 contract the h part (rows 1024..2047), 0..3 the x part.
    ROUNDS = [4, 5, 6, 7, 0, 1, 2, 3]
    res_list = [(l, kp) for l in range(L) for kp in ROUNDS]

    nc = bacc.Bacc("TRN2", target_bir_lowering=False, debug=False,
                   num_devices=N_CORES)

    d_w = nc.dram_tensor("wres", [len(res_list), 128, 2, GCOLS], F8,
                         kind="ExternalInput")
    d_bias = nc.dram_tensor("biasf", [B, L, GCOLS], BF16, kind="ExternalInput")
    d_x0t = nc.dram_tensor("x0t", [128, 8, B], F8, kind="ExternalInput")
    d_h0t = nc.dram_tensor("h0t", [L, 128, 8, B], F8, kind="ExternalInput")
    d_c0 = nc.dram_tensor("c0f", [L, B, 1024], F32, kind="ExternalInput")
    d_wlin = nc.dram_tensor("wlint", [128, 8 * 512], BF16, kind="ExternalInput")
    d_blin = nc.dram_tensor("blinr", [B, 512], F32, kind="ExternalInput")
    d_y = nc.dram_tensor("y", [B, 512], F32, kind="ExternalOutput")

    with tile.TileContext(nc) as tc:
        import contextlib
        with contextlib.ExitStack() as ctx:
            sb = ctx.enter_context(tc.tile_pool(name="sb", bufs=1))
            sb2 = ctx.enter_context(tc.tile_pool(name="sb2", bufs=2))
            psum = ctx.enter_context(tc.tile_pool(name="psum", bufs=1, space="PSUM"))

            w_sb = {}
            for i, (l, kp) in enumerate(res_list):
                t = sb.tile([128, 2, GCOLS], F8, name=f"w_{l}_{kp}")
                nc.sync.dma_start(t[:], d_w[i])
                w_sb[(l, kp)] = t
            bias_sb = sb.tile([B, L, GCOLS], BF16, name="bias_sb")
            nc.sync.dma_start(bias_sb[:], d_bias[:])
            x0t_sb = sb.tile([128, 8, B], F8, name="x0t_sb")
            nc.sync.dma_start(x0t_sb[:], d_x0t[:])
            hT8 = [sb.tile([128, 8, B], F8, name=f"hT8_{l}") for l in range(L)]
            for l in range(L):
                nc.sync.dma_start(hT8[l][:], d_h0t[l])
            c_sb = [sb.tile([B, 1024], F32, name=f"c_{l}") for l in range(L)]
            for l in range(L):
                nc.sync.dma_start(c_sb[l][:], d_c0[l])
            wlin_sb = sb.tile([128, 8 * 512], BF16, name="wlin_sb")
            nc.sync.dma_start(wlin_sb[:], d_wlin[:])

            def stat_ap(l, kp, t):
                if kp >= 4:
                    return hT8[l][:, 2 * (kp - 4):2 * (kp - 4) + 2, :]
                if l == 0 and t == 0:
                    return x0t_sb[:, 2 * kp:2 * kp + 2, :]
                return hT8[1 - l][:, 2 * kp:2 * kp + 2, :]

            def mm_block(ps_half, l, t, half, kps, start, stop):
                # half 0 -> gate cols 0:2048 (i,f); half 1 -> 2048:4096 (g,o)
                for ri, kp in enumerate(kps):
                    w = w_sb[(l, kp)]
                    lhsT = stat_ap(l, kp, t)
                    for n in range(4):
                        c0 = 2048 * half + 512 * n
                        nc.tensor.matmul(
                            ps_half[:, 512 * n:512 * (n + 1)],
                            lhsT,
                            w[:, :, c0:c0 + 512],
                            start=start and ri == 0,
                            stop=stop and ri == len(kps) - 1,
                            perf_mode=DR,
                            skip_group_check=True,
                        )

            rep_cm = tc.For_i(0, reps) if reps > 1 else contextlib.nullcontext()
            with rep_cm:
              for t in range(num_steps):
                for l in range(L):
                    psA = psum.tile([B, 2048], F32, name="gpsA", tag="gpsA")
                    psB = psum.tile([B, 2048], F32, name="gpsB", tag="gpsB")
                    # h-pair rounds first: they only need last step's own-layer h
                    mm_block(psA, l, t, 0, [4, 5, 6, 7], start=True, stop=False)
                    mm_block(psB, l, t, 1, [4, 5, 6, 7], start=True, stop=False)
                    # x-pair rounds wait (via Tile deps) for the new hT8
                    mm_block(psA, l, t, 0, [0, 1, 2, 3], start=False, stop=True)
                    gbA = sb.tile([B, 2048], F32, name="gbA", tag="gbA")
                    nc.vector.tensor_add(gbA[:], psA[:], bias_sb[:, l, 0:2048])
                    s_if = sb.tile([B, 2048], F32, name="s_if", tag="s_if")
                    nc.scalar.activation(s_if[:], gbA[:], AF.Sigmoid,
                                         scale=1.0 / ASCALE)
                    t1 = sb.tile([B, 1024], F32, name="t1", tag="t1")
                    nc.vector.tensor_mul(t1[:], s_if[:, 1024:2048], c_sb[l][:])
                    mm_block(psB, l, t, 1, [0, 1, 2, 3], start=False, stop=True)
                    gbB = sb.tile([B, 2048], F32, name="gbB", tag="gbB")
                    nc.vector.tensor_add(gbB[:], psB[:], bias_sb[:, l, 2048:4096])
                    t_g = sb.tile([B, 1024], F32, name="t_g", tag="t_g")
                    s_o = sb.tile([B, 1024], F32, name="s_o", tag="s_o")
                    nc.scalar.activation(t_g[:], gbB[:, 0:1024], AF.Tanh,
                                         scale=1.0 / ASCALE)
                    nc.scalar.activation(s_o[:], gbB[:, 1024:2048], AF.Sigmoid,
                                         scale=1.0 / ASCALE)
                    t2 = sb.tile([B, 1024], F32, name="t2", tag="t2")
                    nc.vector.tensor_mul(t2[:], s_if[:, 0:1024], t_g[:])
                    nc.vector.tensor_add(c_sb[l][:], t1[:], t2[:])
                    th = sb.tile([B, 1024], F32, name="th", tag="t1")
                    nc.scalar.activation(th[:], c_sb[l][:], AF.Tanh)
                    h16 = sb2.tile([B, 1024], BF16, name="h16", tag="h16")
                    nc.vector.tensor_mul(h16[:], s_o[:], th[:])
                    # rebuild the transposed fp8 stationary for the next step:
                    # DMA XBAR transpose (bf16) then Act Copy quantize (x32)
                    hbT = sb2.tile([128, 8, B], BF16, name="hbT", tag="hbT")
                    nc.sync.dma_start(hbT[:], h16[:], transpose=True)
                    nc.scalar.activation(hT8[l][:], hbT[:], AF.Copy, scale=XSCALE)

            # ---- linear head: pred = h1 @ (W_lin/32).T + b_lin ----
            blin_sb = sb.tile([B, 512], F32, name="blin_sb", tag="t1")
            nc.sync.dma_start(blin_sb[:], d_blin[:])
            ph = psum.tile([B, 2048], F32, name="ph", tag="gpsA")
            for k in range(8):
                lhsT = hT8[1][:, k, :]
                nc.tensor.matmul(ph[:, 0:512], lhsT,
                                 wlin_sb[:, 512 * k:512 * (k + 1)],
                                 start=(k == 0), stop=(k == 7))
            out_sb = sb.tile([B, 512], F32, name="out_sb", tag="s_o")
            nc.vector.tensor_add(out_sb[:], ph[:, 0:512], blin_sb[:])
            nc.sync.dma_start(d_y[:], out_sb[:])

    nc.compile()
    return nc, dict(res_list=res_list)


def _prep_common(inputs, meta):
    W_ih, W_hh = np.asarray(inputs["W_ih"]), np.asarray(inputs["W_hh"])
    b_sum = np.asarray(inputs["b_ih"]) + np.asarray(inputs["b_hh"])
    Wcat = np.concatenate([W_ih, W_hh], axis=2)               # [L, 4096, 2048]
    Wt = np.ascontiguousarray(
        Wcat.reshape(L, GCOLS, 2 * KP, 128).transpose(0, 2, 3, 1)
    )                                                         # [l, k, p, col]
    Wt = (Wt * WSCALE).astype(F8_NP)
    wres = np.stack([
        np.stack([Wt[l, 2 * kp], Wt[l, 2 * kp + 1]], axis=1)
        for (l, kp) in meta["res_list"]
    ])                                                        # [16, 128, 2, 4096]
    bias = np.ascontiguousarray(np.broadcast_to(
        (b_sum * ASCALE).astype(ml_dtypes.bfloat16)[None, :, :], (B, L, GCOLS)))
    W_lin = np.asarray(inputs["W_lin"]) / XSCALE              # cancels h fp8 scale
    wlint = np.ascontiguousarray(
        W_lin.T.reshape(8, 128, 512).transpose(1, 0, 2).reshape(128, 8 * 512)
    ).astype(ml_dtypes.bfloat16)
    blin = np.ascontiguousarray(np.broadcast_to(
        np.asarray(inputs["b_lin"]).astype(np.float32)[None, :], (B, 512)))
    return dict(wres=wres, biasf=bias, wlint=wlint, blinr=blin)


def _prep_core(inputs, ci):
    s = slice(ci * B, (ci + 1) * B)
    x = np.asarray(inputs["input_seq"])[s]
    h = np.asarray(inputs["h"])[:, s]
    c = np.asarray(inputs["c"])[:, s]
    x0t = np.ascontiguousarray(
        (x * XSCALE).reshape(B, 8, 128).transpose(2, 1, 0)
    ).astype(F8_NP)                            # [128, 8, B]
    h0t = np.ascontiguousarray(
        (h * XSCALE).reshape(L, B, 8, 128).transpose(0, 3, 2, 1)
    ).astype(F8_NP)                            # [L, 128, 8, B]
    c0f = np.ascontiguousarray(c).astype(np.float32)   # [L, B, 1024]
    return dict(x0t=x0t, h0t=h0t, c0f=c0f)


_CACHE = {}


def _get_built():
    if "nc" not in _CACHE:
        _CACHE["nc"], _CACHE["meta"] = _build()
    return _CACHE["nc"], _CACHE["meta"]


def kernel(**inputs) -> np.ndarray:
    nc, meta = _get_built()
    common = _prep_common(inputs, meta)
    in_maps = [dict(common, **_prep_core(inputs, ci)) for ci in range(N_CORES)]
    r = bass_utils.run_bass_kernel_spmd(nc, in_maps, core_ids=list(range(N_CORES)))
    y = np.concatenate([r.results[ci]["y"] for ci in range(N_CORES)], axis=0)
    return y.astype(np.float32)
